# revision 7
# baseline (speedup 1.0000x reference)
"""Trainium2 Bass kernel for a decoder layer (DecoderAttention).

Math (reference):
    x   = tok_emb[target_tokens] + pos_emb[:S]                   # [B,S,H]
    x   = attn(x, x,   Wq_s, Wk_s, Wv_s, causal=True)            # self-attn
    x   = attn(x, enc, Wq_c, Wk_c, Wv_c, causal=False)           # cross-attn
    out = x @ Wout + bout                                        # [B,S,V]
with B=4, S=512, ENC=1024, H=1024, V=32000, single-head over full hidden dim.

Sharding: 8 cores, each owns 256 decoder rows (half of one batch; core c ->
batch c//2, row-half c%2).  Per core: compute K/V for its full batch
(duplicated between the pair of cores that share a batch - removes all
collectives), Q/attention for its own 256 rows, and the dominant output
projection (256 x 32000) streaming the full Wout.  All per-core asymmetry
(which rows, causal mask) is staged as input *data* so a single SPMD program
serves all 8 cores.

Host-side prep inside kernel() (layout/sharding only): embedding gather
(tok_emb[tokens] + pos), transposes to [H, seq] layout, causal masks.
"""

import numpy as np

import concourse.bass as bass
import concourse.mybir as mybir
import concourse.tile as tile
from concourse import bacc
from concourse.bass_utils import run_bass_kernel_spmd
from concourse.masks import make_identity

P = 128
B, S, ENC, H, V = 4, 512, 1024, 1024, 32000
HT = H // P            # 8 h-tiles of 128
MYQ = S // 2           # 256 query rows owned per core
QC = MYQ // P          # 2 query chunks of 128
NV = 512               # vocab tile (max fp32 moving free dim)
NVC = (V + NV - 1) // NV   # 63 vocab chunks (62*512 + 1*256)
NCORES = 8
F32 = mybir.dt.float32
SCALE = 1.0 / np.sqrt(H)


def build_program(has_b_s=False, has_b_c=False, has_bout=False,
                  mm_dtype=F32):
    """Trace the single-core SPMD program. Returns nc.

    has_b_s/has_b_c: emit bias adds for self/cross QKV projections.
    has_bout: emit output bias add (via K=1 ones-matmul).
    mm_dtype: dtype used for the big matmul operands (float32 exact;
              float32r runs the PE at full rate with reduced precision).
    """
    # Bacc (not raw Bass): its compile() legalizes multi-wait instructions
    # (move_matmul_waits_to_ldweights + generate_event_semaphores) - walrus
    # rejects >1 sync wait per instruction otherwise.
    nc = bacc.Bacc("TRN2", target_bir_lowering=False, debug=False,
                   num_devices=NCORES)

    xT_d = nc.dram_tensor("xT", [H, S], F32, kind="ExternalInput")
    xTq_d = nc.dram_tensor("xTq", [H, MYQ], F32, kind="ExternalInput")
    encT_d = nc.dram_tensor("encT", [H, ENC], F32, kind="ExternalInput")
    mask_d = nc.dram_tensor("mask", [QC, P, S], F32, kind="ExternalInput")
    wqs_d = nc.dram_tensor("Wq_s", [H, H], F32, kind="ExternalInput")
    wks_d = nc.dram_tensor("Wk_s", [H, H], F32, kind="ExternalInput")
    wvs_d = nc.dram_tensor("Wv_s", [H, H], F32, kind="ExternalInput")
    wqc_d = nc.dram_tensor("Wq_c", [H, H], F32, kind="ExternalInput")
    wkc_d = nc.dram_tensor("Wk_c", [H, H], F32, kind="ExternalInput")
    wvc_d = nc.dram_tensor("Wv_c", [H, H], F32, kind="ExternalInput")
    wout_d = nc.dram_tensor("Wout", [H, V], F32, kind="ExternalInput")
    out_d = nc.dram_tensor("out", [MYQ, V], F32, kind="ExternalOutput")
    if has_b_s:
        bqs_d = nc.dram_tensor("bq_s", [H], F32, kind="ExternalInput")
        bks_d = nc.dram_tensor("bk_s", [H], F32, kind="ExternalInput")
        bvs_d = nc.dram_tensor("bv_s", [H], F32, kind="ExternalInput")
    if has_b_c:
        bqc_d = nc.dram_tensor("bq_c", [H], F32, kind="ExternalInput")
        bkc_d = nc.dram_tensor("bk_c", [H], F32, kind="ExternalInput")
        bvc_d = nc.dram_tensor("bv_c", [H], F32, kind="ExternalInput")
    if has_bout:
        bout_d = nc.dram_tensor("bout", [V], F32, kind="ExternalInput")

    Exp = mybir.ActivationFunctionType.Exp
    AX = mybir.AxisListType.X
    ADD = mybir.AluOpType.add

    def load_bias(pool, b_dram, name):
        # [H] -> SBUF [128, HT]; column ho holds bias[ho*128:(ho+1)*128]
        t = pool.tile([P, HT], F32, name=name)
        nc.sync.dma_start(out=t[:, :], in_=b_dram[:].rearrange("(hi p) -> p hi", p=P))
        return t

    def proj_T(dst_tiles, w_tiles, rhs_tiles, ncols, bias_t):
        """dst[ho] [128, ncols] = (W.T @ rhs)[ho-chunk] (+ bias), all h_out tiles.

        w_tiles[hi]: [128, H] SBUF (h_in chunk hi on partitions).
        rhs_tiles[hi]: [128, ncols] SBUF (h_in on partitions).
        """
        for ho in range(HT):
            ps = psum.tile([P, ncols], F32, tag="acc")
            for hi in range(HT):
                nc.tensor.matmul(
                    out=ps[:, :],
                    lhsT=w_tiles[hi][:, ho * P:(ho + 1) * P],
                    rhs=rhs_tiles[hi][:, :],
                    start=(hi == 0), stop=(hi == HT - 1),
                )
            if bias_t is not None:
                nc.vector.tensor_scalar_add(dst_tiles[ho][:, :], ps[:, :],
                                            bias_t[:, ho:ho + 1])
            else:
                nc.any.tensor_copy(out=dst_tiles[ho][:, :], in_=ps[:, :])

    def softmax_rows(p_sb, s_sb, nkeys):
        """p_sb = softmax(SCALE * s_sb) along free dim. s_sb [128, nkeys] SBUF."""
        mx = stat.tile([P, 1], F32, tag="mx")
        nm = stat.tile([P, 1], F32, tag="nm")
        rs = stat.tile([P, 1], F32, tag="rs")
        ri = stat.tile([P, 1], F32, tag="ri")
        nc.vector.reduce_max(out=mx[:, :], in_=s_sb, axis=AX)
        nc.vector.tensor_scalar_mul(nm[:, :], mx[:, :], -SCALE)
        nc.scalar.activation(p_sb, s_sb, Exp, bias=nm[:, :], scale=SCALE,
                             accum_out=rs[:, :])
        nc.vector.reciprocal(out=ri[:, :], in_=rs[:, :])
        return ri

    with tile.TileContext(nc) as tc:
        with tc.tile_pool(name="persist", bufs=1) as persist, \
             tc.tile_pool(name="stat", bufs=4) as stat, \
             tc.tile_pool(name="psum", bufs=4, space="PSUM") as psum, \
             tc.tile_pool(name="psum_tp", bufs=2, space="PSUM") as psum_tp:

            ident = persist.tile([P, P], F32, name="ident")
            make_identity(nc, ident[:, :])

            att1T = [persist.tile([P, MYQ], F32, name=f"att1T{i}") for i in range(HT)]
            att2T = [persist.tile([P, MYQ], F32, name=f"att2T{i}") for i in range(HT)]

            # ---------------- Phase A: self-attention ----------------
            with tc.tile_pool(name="phA", bufs=1) as pA, \
                 tc.tile_pool(name="wstr", bufs=12) as wpool:

                xT = [pA.tile([P, S], F32, name=f"xT{i}") for i in range(HT)]
                xTq = [pA.tile([P, MYQ], F32, name=f"xTq{i}") for i in range(HT)]
                masks = [pA.tile([P, S], F32, name=f"mask{i}") for i in range(QC)]
                for hi in range(HT):
                    nc.sync.dma_start(out=xT[hi][:, :], in_=xT_d[hi * P:(hi + 1) * P, :])
                    nc.sync.dma_start(out=xTq[hi][:, :], in_=xTq_d[hi * P:(hi + 1) * P, :])
                for qc in range(QC):
                    nc.sync.dma_start(out=masks[qc][:, :], in_=mask_d[qc, :, :])

                bq = bk = bv = None
                if has_b_s:
                    bq = load_bias(pA, bqs_d, "bqs")
                    bk = load_bias(pA, bks_d, "bks")
                    bv = load_bias(pA, bvs_d, "bvs")

                def load_w(w_dram, wname):
                    ts = []
                    for hi in range(HT):
                        t = wpool.tile([P, H], F32, tag="w", name=f"{wname}{hi}")
                        nc.sync.dma_start(out=t[:, :], in_=w_dram[hi * P:(hi + 1) * P, :])
                        ts.append(t)
                    return ts

                # kT[ho] [128, S]; qT[ho] [128, MYQ]; vS[kc] [128 seq, H]
                kT = [pA.tile([P, S], F32, name=f"kT{i}") for i in range(HT)]
                qT = [pA.tile([P, MYQ], F32, name=f"qT{i}") for i in range(HT)]
                vS = [pA.tile([P, H], F32, name=f"vS{i}") for i in range(S // P)]
                pT = [pA.tile([P, MYQ], F32, name=f"pT{i}") for i in range(S // P)]

                wq = load_w(wqs_d, "wqs")
                proj_T(qT, wq, xTq, MYQ, bq)
                wk = load_w(wks_d, "wks")
                proj_T(kT, wk, xT, S, bk)
                wv = load_w(wvs_d, "wvs")
                # v in natural layout [seq, h]: vS[sc] = xT[:, sc-chunk].T @ Wv
                for sc in range(S // P):
                    for hh in range(H // NV):
                        ps = psum.tile([P, NV], F32, tag="acc")
                        for hi in range(HT):
                            nc.tensor.matmul(
                                out=ps[:, :],
                                lhsT=xT[hi][:, sc * P:(sc + 1) * P],
                                rhs=wv[hi][:, hh * NV:(hh + 1) * NV],
                                start=(hi == 0), stop=(hi == HT - 1),
                            )
                        nc.any.tensor_copy(out=vS[sc][:, hh * NV:(hh + 1) * NV],
                                           in_=ps[:, :])

                # scores -> softmax -> p^T, per 128-row query chunk
                for qc in range(QC):
                    sp = psum.tile([P, S], F32, tag="acc")
                    for hi in range(HT):
                        nc.tensor.matmul(
                            out=sp[:, :],
                            lhsT=qT[hi][:, qc * P:(qc + 1) * P],
                            rhs=kT[hi][:, :],
                            start=(hi == 0), stop=(hi == HT - 1),
                        )
                    ssb = pA.tile([P, S], F32, tag="ssb")
                    nc.vector.tensor_tensor(out=ssb[:, :], in0=sp[:, :],
                                            in1=masks[qc][:, :], op=ADD)
                    p_sb = pA.tile([P, S], F32, tag="psb")
                    ri = softmax_rows(p_sb[:, :], ssb[:, :], S)
                    pn = pA.tile([P, S], F32, tag="pn")
                    nc.vector.tensor_scalar_mul(pn[:, :], p_sb[:, :], ri[:, :])
                    for kc in range(S // P):
                        tp = psum_tp.tile([P, P], F32, tag="tp")
                        nc.tensor.transpose(tp[:, :], pn[:, kc * P:(kc + 1) * P],
                                            ident[:, :])
                        nc.any.tensor_copy(out=pT[kc][:, qc * P:(qc + 1) * P],
                                           in_=tp[:, :])

                # att1^T[ho] = sum_kc vS[kc][:, ho-chunk].T @ pT[kc]
                for ho in range(HT):
                    ps = psum.tile([P, MYQ], F32, tag="acc")
                    for kc in range(S // P):
                        nc.tensor.matmul(
                            out=ps[:, :],
                            lhsT=vS[kc][:, ho * P:(ho + 1) * P],
                            rhs=pT[kc][:, :],
                            start=(kc == 0), stop=(kc == S // P - 1),
                        )
                    if bv is not None:
                        nc.vector.tensor_scalar_add(att1T[ho][:, :], ps[:, :],
                                                    bv[:, ho:ho + 1])
                    else:
                        nc.any.tensor_copy(out=att1T[ho][:, :], in_=ps[:, :])

            # ---------------- Phase B: cross-attention ----------------
            with tc.tile_pool(name="phB", bufs=1) as pB, \
                 tc.tile_pool(name="wstr2", bufs=12) as wpool2:

                encT = [pB.tile([P, ENC], F32, name=f"encT{i}") for i in range(HT)]
                for hi in range(HT):
                    nc.sync.dma_start(out=encT[hi][:, :],
                                      in_=encT_d[hi * P:(hi + 1) * P, :])

                bq = bk = bv = None
                if has_b_c:
                    bq = load_bias(pB, bqc_d, "bqc")
                    bk = load_bias(pB, bkc_d, "bkc")
                    bv = load_bias(pB, bvc_d, "bvc")

                def load_w2(w_dram, wname):
                    ts = []
                    for hi in range(HT):
                        t = wpool2.tile([P, H], F32, tag="w2", name=f"{wname}{hi}")
                        nc.sync.dma_start(out=t[:, :], in_=w_dram[hi * P:(hi + 1) * P, :])
                        ts.append(t)
                    return ts

                q2T = [pB.tile([P, MYQ], F32, name=f"q2T{i}") for i in range(HT)]
                k2T = [pB.tile([P, ENC], F32, name=f"k2T{i}") for i in range(HT)]
                v2 = [pB.tile([P, H], F32, name=f"v2_{i}") for i in range(ENC // P)]
                p2T = [pB.tile([P, MYQ], F32, name=f"p2T{i}") for i in range(ENC // P)]

                wq = load_w2(wqc_d, "wqc")
                proj_T(q2T, wq, att1T, MYQ, bq)
                wk = load_w2(wkc_d, "wkc")
                # k2T[ho] [128, ENC=1024]: two 512-wide halves
                for ho in range(HT):
                    for eh in range(ENC // NV):
                        ps = psum.tile([P, NV], F32, tag="acc")
                        for hi in range(HT):
                            nc.tensor.matmul(
                                out=ps[:, :],
                                lhsT=wk[hi][:, ho * P:(ho + 1) * P],
                                rhs=encT[hi][:, eh * NV:(eh + 1) * NV],
                                start=(hi == 0), stop=(hi == HT - 1),
                            )
                        if bk is not None:
                            nc.vector.tensor_scalar_add(
                                k2T[ho][:, eh * NV:(eh + 1) * NV], ps[:, :],
                                bk[:, ho:ho + 1])
                        else:
                            nc.any.tensor_copy(out=k2T[ho][:, eh * NV:(eh + 1) * NV],
                                               in_=ps[:, :])
                wv = load_w2(wvc_d, "wvc")
                for ec in range(ENC // P):
                    for hh in range(H // NV):
                        ps = psum.tile([P, NV], F32, tag="acc")
                        for hi in range(HT):
                            nc.tensor.matmul(
                                out=ps[:, :],
                                lhsT=encT[hi][:, ec * P:(ec + 1) * P],
                                rhs=wv[hi][:, hh * NV:(hh + 1) * NV],
                                start=(hi == 0), stop=(hi == HT - 1),
                            )
                        nc.any.tensor_copy(out=v2[ec][:, hh * NV:(hh + 1) * NV],
                                           in_=ps[:, :])

                for qc in range(QC):
                    s2 = pB.tile([P, ENC], F32, tag="s2")
                    for eh in range(ENC // NV):
                        sp = psum.tile([P, NV], F32, tag="acc")
                        for hi in range(HT):
                            nc.tensor.matmul(
                                out=sp[:, :],
                                lhsT=q2T[hi][:, qc * P:(qc + 1) * P],
                                rhs=k2T[hi][:, eh * NV:(eh + 1) * NV],
                                start=(hi == 0), stop=(hi == HT - 1),
                            )
                        nc.any.tensor_copy(out=s2[:, eh * NV:(eh + 1) * NV],
                                           in_=sp[:, :])
                    p_sb = pB.tile([P, ENC], F32, tag="p2sb")
                    ri = softmax_rows(p_sb[:, :], s2[:, :], ENC)
                    pn = pB.tile([P, ENC], F32, tag="p2n")
                    nc.vector.tensor_scalar_mul(pn[:, :], p_sb[:, :], ri[:, :])
                    for ec in range(ENC // P):
                        tp = psum_tp.tile([P, P], F32, tag="tp")
                        nc.tensor.transpose(tp[:, :], pn[:, ec * P:(ec + 1) * P],
                                            ident[:, :])
                        nc.any.tensor_copy(out=p2T[ec][:, qc * P:(qc + 1) * P],
                                           in_=tp[:, :])

                for ho in range(HT):
                    ps = psum.tile([P, MYQ], F32, tag="acc")
                    for ec in range(ENC // P):
                        nc.tensor.matmul(
                            out=ps[:, :],
                            lhsT=v2[ec][:, ho * P:(ho + 1) * P],
                            rhs=p2T[ec][:, :],
                            start=(ec == 0), stop=(ec == ENC // P - 1),
                        )
                    if bv is not None:
                        nc.vector.tensor_scalar_add(att2T[ho][:, :], ps[:, :],
                                                    bv[:, ho:ho + 1])
                    else:
                        nc.any.tensor_copy(out=att2T[ho][:, :], in_=ps[:, :])

            # ---------------- Phase C: output projection ----------------
            with tc.tile_pool(name="phC_w", bufs=3) as pW, \
                 tc.tile_pool(name="phC_o", bufs=6) as pO:
                psC = psum

                ones_t = None
                if has_bout:
                    ones_t = persist.tile([1, P], F32, name="ones")
                    nc.vector.memset(ones_t[:, :], 1.0)

                a2 = att2T
                if mm_dtype != F32:
                    a2 = [persist.tile([P, MYQ], mm_dtype, name=f"a2c{i}")
                          for i in range(HT)]
                    for hi in range(HT):
                        nc.vector.tensor_copy(out=a2[hi][:, :], in_=att2T[hi][:, :])

                for vc in range(NVC):
                    nv = min(NV, V - vc * NV)
                    wt = pW.tile([P, HT, nv], mm_dtype, tag="wt")
                    nc.sync.dma_start(
                        out=wt[:, :, :],
                        in_=wout_d[:, vc * NV:vc * NV + nv].rearrange(
                            "(hi p) j -> p hi j", p=P),
                    )
                    bo = None
                    if has_bout:
                        bo = pW.tile([1, nv], F32, tag="bo")
                        nc.sync.dma_start(out=bo[:, :],
                                          in_=bout_d[vc * NV:vc * NV + nv][None, :])
                    for qc in range(QC):
                        ps = psC.tile([P, nv], F32, tag="acc")
                        for hi in range(HT):
                            last = (hi == HT - 1) and not has_bout
                            nc.tensor.matmul(
                                out=ps[:, :],
                                lhsT=a2[hi][:, qc * P:(qc + 1) * P],
                                rhs=wt[:, hi, :],
                                start=(hi == 0), stop=last,
                            )
                        if has_bout:
                            nc.tensor.matmul(
                                out=ps[:, :], lhsT=ones_t[:, :], rhs=bo[:, :],
                                start=False, stop=True,
                            )
                        osb = pO.tile([P, NV], F32, tag="osb")
                        nc.any.tensor_copy(out=osb[:, :nv], in_=ps[:, :])
                        nc.sync.dma_start(
                            out=out_d[qc * P:(qc + 1) * P, vc * NV:vc * NV + nv],
                            in_=osb[:, :nv],
                        )
    nc.compile()
    return nc


def _host_prep(inputs):
    """Numpy-side sharding/layout prep. Returns (in_maps, meta)."""
    enc = np.ascontiguousarray(np.asarray(inputs["encoder_outputs"], dtype=np.float32))
    tok = np.asarray(inputs["target_tokens"]).astype(np.int64)
    tok_emb = np.asarray(inputs["tok_emb"], dtype=np.float32)
    pos_emb = np.asarray(inputs["pos_emb"], dtype=np.float32)
    x0 = tok_emb[tok] + pos_emb[:S][None, :, :]          # [B,S,H]
    xT = np.ascontiguousarray(x0.transpose(0, 2, 1))      # [B,H,S]
    encT = np.ascontiguousarray(enc.transpose(0, 2, 1))   # [B,H,ENC]

    ws = {k: np.ascontiguousarray(np.asarray(inputs[k], dtype=np.float32))
          for k in ("Wq_s", "Wk_s", "Wv_s", "Wq_c", "Wk_c", "Wv_c", "Wout")}
    bs = {k: np.asarray(inputs[k], dtype=np.float32)
          for k in ("bq_s", "bk_s", "bv_s", "bq_c", "bk_c", "bv_c", "bout")}
    has_b_s = any(np.any(bs[k]) for k in ("bq_s", "bk_s", "bv_s"))
    has_b_c = any(np.any(bs[k]) for k in ("bq_c", "bk_c", "bv_c"))
    has_bout = bool(np.any(bs["bout"]))

    # additive causal masks per row-half: mask[qc, i, j] = 0 if j <= q0+qc*128+i
    j = np.arange(S)[None, None, :]
    in_maps = []
    for c in range(NCORES):
        b, half = c // 2, c % 2
        q0 = half * MYQ
        i_glob = q0 + np.arange(MYQ).reshape(QC, P)[:, :, None]
        mask = np.where(j <= i_glob, 0.0, -1e9).astype(np.float32)
        m = {
            "xT": xT[b],
            "xTq": np.ascontiguousarray(xT[b][:, q0:q0 + MYQ]),
            "encT": encT[b],
            "mask": mask,
            **{k: ws[k] for k in ws},
        }
        if has_b_s:
            m.update({k: bs[k] for k in ("bq_s", "bk_s", "bv_s")})
        if has_b_c:
            m.update({k: bs[k] for k in ("bq_c", "bk_c", "bv_c")})
        if has_bout:
            m["bout"] = bs["bout"]
        in_maps.append(m)
    return in_maps, (has_b_s, has_b_c, has_bout)


def kernel(**inputs):
    in_maps, (has_b_s, has_b_c, has_bout) = _host_prep(inputs)
    nc = build_program(has_b_s=has_b_s, has_b_c=has_b_c, has_bout=has_bout)
    res = run_bass_kernel_spmd(nc, in_maps, list(range(NCORES)))
    out = np.empty((B, S, V), dtype=np.float32)
    for c in range(NCORES):
        b, half = c // 2, c % 2
        out[b, half * MYQ:(half + 1) * MYQ, :] = res.results[c]["out"]
    return out


# revision 12
# speedup vs baseline: 2.3205x; 2.3205x over previous
"""Trainium2 Bass kernel for a decoder layer (DecoderAttention).

Math (reference):
    x   = tok_emb[target_tokens] + pos_emb[:S]                   # [B,S,H]
    x   = attn(x, x,   Wq_s, Wk_s, Wv_s, causal=True)            # self-attn
    x   = attn(x, enc, Wq_c, Wk_c, Wv_c, causal=False)           # cross-attn
    out = x @ Wout + bout                                        # [B,S,V]
with B=4, S=512, ENC=1024, H=1024, V=32000, single-head over full hidden dim.

Sharding: 8 cores = 4 batches x 2 vocab halves, zero collectives.  Core c
owns batch c//2: it computes the full attention stack for its batch
(duplicated between the pair of cores sharing the batch - cheaper than any
collective here) and the output projection for vocab half c%2 (Wout is the
dominant DMA stream; halving it keeps DMA under the PE time).

Matmuls run in float32r - the PE's single-pass fp32 mode (1 cycle/row at
free dim >= 256 vs 4 for exact fp32; HW-measured 3.3x, rel err ~1.5e-4 per
1024-deep dot product).  The softmax path stays exact fp32.

Host-side prep inside kernel() (layout/sharding only): embedding gather
(tok_emb[tokens] + pos), transposes to [H, seq] layout, causal mask.
"""

import numpy as np

import concourse.mybir as mybir
import concourse.tile as tile
from concourse import bacc, bass
from concourse.bass_utils import run_bass_kernel_spmd
from concourse.masks import make_identity

P = 128
B, S, ENC, H, V = 4, 512, 1024, 1024, 32000
HT = H // P            # 8 h-tiles of 128
SC = S // P            # 4 seq chunks of 128
EC = ENC // P          # 8 encoder chunks
VSH = V // 2           # 16000 vocab columns per core
NV = 500               # vocab tile: 32*500 = 16000, all >= 256 (f32r full rate)
NVC = VSH // NV        # 32
NCORES = 8
F32 = mybir.dt.float32
F32R = mybir.dt.float32r
SCALE = 1.0 / np.sqrt(H)


def build_program(has_b_s=False, has_b_c=False, has_bout=False, use_f32r=True):
    """Trace + compile the single-core SPMD program. Returns nc."""
    # Bacc (not raw Bass): its compile() legalizes multi-wait instructions
    # (move_matmul_waits_to_ldweights + generate_event_semaphores) - walrus
    # rejects >1 sync wait per instruction otherwise.
    nc = bacc.Bacc("TRN2", target_bir_lowering=False, debug=False,
                   num_devices=NCORES)

    MMDT = F32R if use_f32r else F32

    xT_d = nc.dram_tensor("xT", [H, S], MMDT, kind="ExternalInput")
    encT_d = nc.dram_tensor("encT", [H, ENC], MMDT, kind="ExternalInput")
    mask_d = nc.dram_tensor("mask", [SC, P, S], F32, kind="ExternalInput")
    wqs_d = nc.dram_tensor("Wq_s", [H, H], MMDT, kind="ExternalInput")
    wks_d = nc.dram_tensor("Wk_s", [H, H], MMDT, kind="ExternalInput")
    wvs_d = nc.dram_tensor("Wv_s", [H, H], MMDT, kind="ExternalInput")
    wqc_d = nc.dram_tensor("Wq_c", [H, H], MMDT, kind="ExternalInput")
    wkc_d = nc.dram_tensor("Wk_c", [H, H], MMDT, kind="ExternalInput")
    wvc_d = nc.dram_tensor("Wv_c", [H, H], MMDT, kind="ExternalInput")
    wout_d = nc.dram_tensor("Wout", [H, VSH], MMDT, kind="ExternalInput")
    out_d = nc.dram_tensor("out", [S, VSH], F32, kind="ExternalOutput")
    if has_b_s:
        bqs_d = nc.dram_tensor("bq_s", [H], F32, kind="ExternalInput")
        bks_d = nc.dram_tensor("bk_s", [H], F32, kind="ExternalInput")
        bvs_d = nc.dram_tensor("bv_s", [H], F32, kind="ExternalInput")
    if has_b_c:
        bqc_d = nc.dram_tensor("bq_c", [H], F32, kind="ExternalInput")
        bkc_d = nc.dram_tensor("bk_c", [H], F32, kind="ExternalInput")
        bvc_d = nc.dram_tensor("bv_c", [H], F32, kind="ExternalInput")
    if has_bout:
        bout_d = nc.dram_tensor("bout", [VSH], F32, kind="ExternalInput")

    Exp = mybir.ActivationFunctionType.Exp
    AX = mybir.AxisListType.X
    ADD = mybir.AluOpType.add

    def load_bias(pool, b_dram, name):
        # [H] -> SBUF [128, HT]; column ho holds bias[ho*128:(ho+1)*128]
        t = pool.tile([P, HT], F32, name=name)
        nc.sync.dma_start(out=t[:, :], in_=b_dram[:].rearrange("(hi p) -> p hi", p=P))
        return t

    with tile.TileContext(nc) as tc:
        with tc.tile_pool(name="persist", bufs=1) as persist, \
             tc.tile_pool(name="stat", bufs=4) as stat, \
             tc.tile_pool(name="smx", bufs=1) as smx, \
             tc.tile_pool(name="att1p", bufs=1) as att1p, \
             tc.tile_pool(name="psum", bufs=4, space="PSUM") as psum, \
             tc.tile_pool(name="psum_tp", bufs=2, space="PSUM") as psum_tp:

            ident = persist.tile([P, P], F32, name="ident")
            make_identity(nc, ident[:, :])

            # att1T/att2T: [h, seq] activations consumed by later matmuls
            att1T = [att1p.tile([P, S], MMDT, name=f"att1T{i}") for i in range(HT)]
            att2T = [persist.tile([P, S], MMDT, name=f"att2T{i}") for i in range(HT)]

            def load_w(w_dram, wname, pool, tag):
                """Weight [H, H] as 16 half-tiles [(hi, half)] of [128, 512].

                hf-major load order: consumers use a full (hi=0..7, hf) set per
                output chunk, so all of hf=0 must be resident before any of
                hf=1 claims a slot (hi-major order deadlocks the slot pool).
                """
                ts = [[None, None] for _ in range(HT)]
                for hf in range(2):
                    for hi in range(HT):
                        t = pool.tile([P, 512], MMDT, tag=tag,
                                      name=f"{wname}{hi}_{hf}")
                        nc.sync.dma_start(
                            out=t[:, :],
                            in_=w_dram[hi * P:(hi + 1) * P, hf * 512:(hf + 1) * 512])
                        ts[hi][hf] = t
                return ts

            def wcol(w, hi, ho):
                # lhsT [128, 128] slice for h_out chunk ho from half-tiles
                return w[hi][ho // 4][:, (ho % 4) * P:(ho % 4 + 1) * P]

            def proj_T(dst_tiles, w_tiles, rhs_tiles, bias_t):
                """dst[ho][128, S] = (W.T @ rhs)[ho-chunk] (+ bias)."""
                for ho in range(HT):
                    ps = psum.tile([P, S], F32, tag="acc")
                    for hi in range(HT):
                        nc.tensor.matmul(
                            out=ps[:, :],
                            lhsT=wcol(w_tiles, hi, ho),
                            rhs=rhs_tiles[hi][:, :],
                            start=(hi == 0), stop=(hi == HT - 1),
                        )
                    if bias_t is not None:
                        nc.vector.tensor_scalar_add(dst_tiles[ho][:, :], ps[:, :],
                                                    bias_t[:, ho:ho + 1])
                    else:
                        nc.vector.tensor_copy(out=dst_tiles[ho][:, :], in_=ps[:, :])

            def softmax_rows(p_sb, s_sb):
                """p_sb = exp(SCALE*(s_sb - rowmax)); returns 1/rowsum [128,1]."""
                mx = stat.tile([P, 1], F32, tag="mx")
                nm = stat.tile([P, 1], F32, tag="nm")
                rs = stat.tile([P, 1], F32, tag="rs")
                ri = stat.tile([P, 1], F32, tag="ri")
                nc.vector.reduce_max(out=mx[:, :], in_=s_sb, axis=AX)
                nc.vector.tensor_scalar_mul(nm[:, :], mx[:, :], -SCALE)
                nc.scalar.activation(p_sb, s_sb, Exp, bias=nm[:, :], scale=SCALE,
                                     accum_out=rs[:, :])
                nc.vector.reciprocal(out=ri[:, :], in_=rs[:, :])
                return ri

            # ---------------- Phase A: self-attention ----------------
            with tc.tile_pool(name="phA", bufs=1) as pA, \
                 tc.tile_pool(name="wstr", bufs=12) as wpool:

                xT = [pA.tile([P, S], MMDT, name=f"xT{i}") for i in range(HT)]
                masks = [pA.tile([P, S], F32, name=f"mask{i}") for i in range(SC)]
                for hi in range(HT):
                    nc.sync.dma_start(out=xT[hi][:, :], in_=xT_d[hi * P:(hi + 1) * P, :])
                for qc in range(SC):
                    nc.sync.dma_start(out=masks[qc][:, :], in_=mask_d[qc, :, :])

                bq = bk = bv = None
                if has_b_s:
                    bq = load_bias(pA, bqs_d, "bqs")
                    bk = load_bias(pA, bks_d, "bks")
                    bv = load_bias(pA, bvs_d, "bvs")

                kT = [pA.tile([P, S], MMDT, name=f"kT{i}") for i in range(HT)]
                qT = [pA.tile([P, S], MMDT, name=f"qT{i}") for i in range(HT)]
                vS = [pA.tile([P, H], MMDT, name=f"vS{i}") for i in range(SC)]
                pT = [pA.tile([P, S], MMDT, name=f"pT{i}") for i in range(SC)]

                wq = load_w(wqs_d, "wqs", wpool, "w")
                proj_T(qT, wq, xT, bq)
                wk = load_w(wks_d, "wks", wpool, "w")
                proj_T(kT, wk, xT, bk)
                wv = load_w(wvs_d, "wvs", wpool, "w")
                # v in natural layout [seq, h]: vS[sc] = xT[:, sc-chunk].T @ Wv
                for hh in range(2):
                    for sc in range(SC):
                        ps = psum.tile([P, 512], F32, tag="acc")
                        for hi in range(HT):
                            nc.tensor.matmul(
                                out=ps[:, :],
                                lhsT=xT[hi][:, sc * P:(sc + 1) * P],
                                rhs=wv[hi][hh][:, :],
                                start=(hi == 0), stop=(hi == HT - 1),
                            )
                        nc.vector.tensor_copy(out=vS[sc][:, hh * 512:(hh + 1) * 512],
                                              in_=ps[:, :])

                # scores -> softmax -> p^T, per 128-row query chunk
                for qc in range(SC):
                    sp = psum.tile([P, S], F32, tag="acc")
                    for hi in range(HT):
                        nc.tensor.matmul(
                            out=sp[:, :],
                            lhsT=qT[hi][:, qc * P:(qc + 1) * P],
                            rhs=kT[hi][:, :],
                            start=(hi == 0), stop=(hi == HT - 1),
                        )
                    ssb = smx.tile([P, S], F32, tag="ssb")
                    nc.vector.tensor_tensor(out=ssb[:, :], in0=sp[:, :],
                                            in1=masks[qc][:, :], op=ADD)
                    p_sb = smx.tile([P, S], F32, tag="psb")
                    ri = softmax_rows(p_sb[:, :], ssb[:, :])
                    pn = smx.tile([P, S], F32, tag="pn")
                    nc.vector.tensor_scalar_mul(pn[:, :], p_sb[:, :], ri[:, :])
                    for kc in range(SC):
                        tp = psum_tp.tile([P, P], F32, tag="tp")
                        nc.tensor.transpose(tp[:, :], pn[:, kc * P:(kc + 1) * P],
                                            ident[:, :])
                        nc.vector.tensor_copy(out=pT[kc][:, qc * P:(qc + 1) * P],
                                              in_=tp[:, :])

                # att1^T[ho] = sum_kc vS[kc][:, ho-chunk].T @ pT[kc]
                for ho in range(HT):
                    ps = psum.tile([P, S], F32, tag="acc")
                    for kc in range(SC):
                        nc.tensor.matmul(
                            out=ps[:, :],
                            lhsT=vS[kc][:, ho * P:(ho + 1) * P],
                            rhs=pT[kc][:, :],
                            start=(kc == 0), stop=(kc == SC - 1),
                        )
                    if bv is not None:
                        nc.vector.tensor_scalar_add(att1T[ho][:, :], ps[:, :],
                                                    bv[:, ho:ho + 1])
                    else:
                        nc.vector.tensor_copy(out=att1T[ho][:, :], in_=ps[:, :])

            # ---------------- Phase B: cross-attention ----------------
            with tc.tile_pool(name="phB", bufs=1) as pB, \
                 tc.tile_pool(name="wstr2", bufs=12) as wpool2:

                bq = bk = bv = None
                if has_b_c:
                    bq = load_bias(pB, bqc_d, "bqc")
                    bk = load_bias(pB, bkc_d, "bkc")
                    bv = load_bias(pB, bvc_d, "bvc")

                k2T = [pB.tile([P, ENC], MMDT, name=f"k2T{i}") for i in range(HT)]
                v2 = [pB.tile([P, H], MMDT, name=f"v2_{i}") for i in range(EC)]

                # encT lives only while k2/v2 are computed
                with tc.tile_pool(name="phBenc", bufs=1) as pBe:
                    encT = [pBe.tile([P, ENC], MMDT, name=f"encT{i}")
                            for i in range(HT)]
                    for hi in range(HT):
                        nc.sync.dma_start(out=encT[hi][:, :],
                                          in_=encT_d[hi * P:(hi + 1) * P, :])

                    wk = load_w(wkc_d, "wkc", wpool2, "w2")
                    # k2T[ho] [128, ENC=1024]: two 512-wide halves
                    for ho in range(HT):
                        for eh in range(2):
                            ps = psum.tile([P, 512], F32, tag="acc")
                            for hi in range(HT):
                                nc.tensor.matmul(
                                    out=ps[:, :],
                                    lhsT=wcol(wk, hi, ho),
                                    rhs=encT[hi][:, eh * 512:(eh + 1) * 512],
                                    start=(hi == 0), stop=(hi == HT - 1),
                                )
                            if bk is not None:
                                nc.vector.tensor_scalar_add(
                                    k2T[ho][:, eh * 512:(eh + 1) * 512], ps[:, :],
                                    bk[:, ho:ho + 1])
                            else:
                                nc.vector.tensor_copy(
                                    out=k2T[ho][:, eh * 512:(eh + 1) * 512],
                                    in_=ps[:, :])
                    wv = load_w(wvc_d, "wvc", wpool2, "w2")
                    for hh in range(2):
                        for ec in range(EC):
                            ps = psum.tile([P, 512], F32, tag="acc")
                            for hi in range(HT):
                                nc.tensor.matmul(
                                    out=ps[:, :],
                                    lhsT=encT[hi][:, ec * P:(ec + 1) * P],
                                    rhs=wv[hi][hh][:, :],
                                    start=(hi == 0), stop=(hi == HT - 1),
                                )
                            nc.vector.tensor_copy(
                                out=v2[ec][:, hh * 512:(hh + 1) * 512], in_=ps[:, :])

                q2T = [pB.tile([P, S], MMDT, name=f"q2T{i}") for i in range(HT)]
                p2T = [pB.tile([P, S], MMDT, name=f"p2T{i}") for i in range(EC)]

                wq = load_w(wqc_d, "wqc", wpool2, "w2")
                proj_T(q2T, wq, att1T, bq)

                for qc in range(SC):
                    s2 = smx.tile([P, ENC], F32, tag="s2")
                    for eh in range(2):
                        sp = psum.tile([P, 512], F32, tag="acc")
                        for hi in range(HT):
                            nc.tensor.matmul(
                                out=sp[:, :],
                                lhsT=q2T[hi][:, qc * P:(qc + 1) * P],
                                rhs=k2T[hi][:, eh * 512:(eh + 1) * 512],
                                start=(hi == 0), stop=(hi == HT - 1),
                            )
                        nc.vector.tensor_copy(out=s2[:, eh * 512:(eh + 1) * 512],
                                              in_=sp[:, :])
                    p_sb = smx.tile([P, ENC], F32, tag="p2sb")
                    ri = softmax_rows(p_sb[:, :], s2[:, :])
                    pn = smx.tile([P, ENC], F32, tag="p2n")
                    nc.vector.tensor_scalar_mul(pn[:, :], p_sb[:, :], ri[:, :])
                    for ec in range(EC):
                        tp = psum_tp.tile([P, P], F32, tag="tp")
                        nc.tensor.transpose(tp[:, :], pn[:, ec * P:(ec + 1) * P],
                                            ident[:, :])
                        nc.vector.tensor_copy(out=p2T[ec][:, qc * P:(qc + 1) * P],
                                              in_=tp[:, :])

                for ho in range(HT):
                    ps = psum.tile([P, S], F32, tag="acc")
                    for ec in range(EC):
                        nc.tensor.matmul(
                            out=ps[:, :],
                            lhsT=v2[ec][:, ho * P:(ho + 1) * P],
                            rhs=p2T[ec][:, :],
                            start=(ec == 0), stop=(ec == EC - 1),
                        )
                    if bv is not None:
                        nc.vector.tensor_scalar_add(att2T[ho][:, :], ps[:, :],
                                                    bv[:, ho:ho + 1])
                    else:
                        nc.vector.tensor_copy(out=att2T[ho][:, :], in_=ps[:, :])

            # ---------------- Phase C: output projection ----------------
            with tc.tile_pool(name="phC_w", bufs=3) as pW, \
                 tc.tile_pool(name="phC_o", bufs=8) as pO:

                ones_t = None
                if has_bout:
                    ones_t = persist.tile([1, P], MMDT, name="ones")
                    nc.vector.memset(ones_t[:, :], 1.0)

                for vc in range(NVC):
                    wt = pW.tile([P, HT, NV], MMDT, tag="wt")
                    nc.sync.dma_start(
                        out=wt[:, :, :],
                        in_=wout_d[:, vc * NV:(vc + 1) * NV].rearrange(
                            "(hi p) j -> p hi j", p=P),
                    )
                    bo = None
                    if has_bout:
                        bo = pW.tile([1, NV], MMDT, tag="bo")
                        nc.gpsimd.dma_start(out=bo[:, :],
                                            in_=bout_d[vc * NV:(vc + 1) * NV][None, :])
                    for qc in range(SC):
                        ps = psum.tile([P, NV], F32, tag="acc")
                        for hi in range(HT):
                            last = (hi == HT - 1) and not has_bout
                            nc.tensor.matmul(
                                out=ps[:, :],
                                lhsT=att2T[hi][:, qc * P:(qc + 1) * P],
                                rhs=wt[:, hi, :],
                                start=(hi == 0), stop=last,
                            )
                        if has_bout:
                            nc.tensor.matmul(
                                out=ps[:, :], lhsT=ones_t[:, :], rhs=bo[:, :],
                                start=False, stop=True,
                            )
                        osb = pO.tile([P, NV], F32, tag="osb")
                        nc.any.tensor_copy(out=osb[:, :], in_=ps[:, :])
                        nc.sync.dma_start(
                            out=out_d[qc * P:(qc + 1) * P, vc * NV:(vc + 1) * NV],
                            in_=osb[:, :],
                        )
    nc.compile()
    return nc


def _host_prep(inputs):
    """Numpy-side sharding/layout prep. Returns (in_maps, flags)."""
    enc = np.asarray(inputs["encoder_outputs"], dtype=np.float32)
    tok = np.asarray(inputs["target_tokens"]).astype(np.int64)
    tok_emb = np.asarray(inputs["tok_emb"], dtype=np.float32)
    pos_emb = np.asarray(inputs["pos_emb"], dtype=np.float32)
    x0 = tok_emb[tok] + pos_emb[:S][None, :, :]          # [B,S,H]
    xT = np.ascontiguousarray(x0.transpose(0, 2, 1))      # [B,H,S]
    encT = np.ascontiguousarray(enc.transpose(0, 2, 1))   # [B,H,ENC]

    ws = {k: np.ascontiguousarray(np.asarray(inputs[k], dtype=np.float32))
          for k in ("Wq_s", "Wk_s", "Wv_s", "Wq_c", "Wk_c", "Wv_c")}
    wout = np.ascontiguousarray(np.asarray(inputs["Wout"], dtype=np.float32))
    bs = {k: np.asarray(inputs[k], dtype=np.float32)
          for k in ("bq_s", "bk_s", "bv_s", "bq_c", "bk_c", "bv_c", "bout")}
    has_b_s = any(np.any(bs[k]) for k in ("bq_s", "bk_s", "bv_s"))
    has_b_c = any(np.any(bs[k]) for k in ("bq_c", "bk_c", "bv_c"))
    has_bout = bool(np.any(bs["bout"]))

    # additive causal mask: mask[qc, i, j] = 0 if j <= qc*128+i else -1e9
    j = np.arange(S)[None, None, :]
    i_glob = np.arange(S).reshape(SC, P)[:, :, None]
    mask = np.where(j <= i_glob, 0.0, -1e9).astype(np.float32)

    in_maps = []
    for c in range(NCORES):
        b, vh = c // 2, c % 2
        m = {
            "xT": xT[b],
            "encT": encT[b],
            "mask": mask,
            **ws,
            "Wout": np.ascontiguousarray(wout[:, vh * VSH:(vh + 1) * VSH]),
        }
        if has_b_s:
            m.update({k: bs[k] for k in ("bq_s", "bk_s", "bv_s")})
        if has_b_c:
            m.update({k: bs[k] for k in ("bq_c", "bk_c", "bv_c")})
        if has_bout:
            m["bout"] = np.ascontiguousarray(bs["bout"][vh * VSH:(vh + 1) * VSH])
        in_maps.append(m)
    return in_maps, (has_b_s, has_b_c, has_bout)


def assemble_output(results):
    out = np.empty((B, S, V), dtype=np.float32)
    for c in range(NCORES):
        b, vh = c // 2, c % 2
        out[b, :, vh * VSH:(vh + 1) * VSH] = results[c]["out"]
    return out


def kernel(**inputs):
    in_maps, (has_b_s, has_b_c, has_bout) = _host_prep(inputs)
    nc = build_program(has_b_s=has_b_s, has_b_c=has_b_c, has_bout=has_bout)
    res = run_bass_kernel_spmd(nc, in_maps, list(range(NCORES)))
    return assemble_output(res.results)
